# revision 58
# baseline (speedup 1.0000x reference)
"""Trainium2 Bass kernel for nn_FELDMSTM_7988639171122 (8 NeuronCores).

v5: v4 + FWL-friendly 128-col stationaries (fast weight load), s_b
stationary-reuse ordering, and merged 16-col mixing matmuls.

Math per (b, n) sample x_s [L=96, D=32]:
  P_s = C6.T @ x_s  [6, 32];  Z = per-node complex mixing of P via U1p/U2p;
  res = M_res @ x_s + G @ Z.
"""

import numpy as np
import ml_dtypes
import concourse.bass as bass
import concourse.bacc as bacc
import concourse.mybir as mybir
from concourse.tile import TileContext

F32 = mybir.dt.float32
F16 = mybir.dt.float16
BF16 = mybir.dt.bfloat16
NP_BF16 = ml_dtypes.bfloat16

L, D, H, E, MODES, KAVG = 96, 32, 4, 8, (1, 4, 5), 25
NB = 8          # batch
NNODE = 250     # nodes per core
NCORES = 8
NT = 16         # nodes per device tile
TD = L * D      # 3072
NTILES = (NNODE + NT - 1) // NT
BDC = 24 * 128  # bd-weight cols per tile: 4 quads x 3 modes x 2 mats x 128


def _host_constants():
    t = np.arange(L)
    th = 2 * np.pi * np.outer(t, np.array(MODES)) / L
    C6 = np.concatenate([np.cos(th), -np.sin(th)], axis=1)
    pad = (KAVG - 1) // 2
    A = np.zeros((L, L))
    for tt in range(L):
        for w in range(KAVG):
            A[tt, min(max(tt + w - pad, 0), L - 1)] += 1.0 / KAVG
    M_res = np.eye(L) - A
    Cinv = np.concatenate([(2.0 / L) * np.cos(th), -(2.0 / L) * np.sin(th)], axis=1)
    G = M_res @ Cinv
    return C6, M_res, G


def _host_node_weights(W1, W2, Wq, Wo):
    N = W1.shape[0]
    WoT = Wo.T.reshape(H, E, D)
    U1 = np.einsum("nheom,hod->nmhed", W1, WoT).reshape(N, 3, H * E, D)
    U2 = np.einsum("nheom,hod->nmhed", W2, WoT).reshape(N, 3, H * E, D)
    U1p = np.einsum("hd,nmhe->nmde", Wq.reshape(H * E, D), U1)
    U2p = np.einsum("hd,nmhe->nmde", Wq.reshape(H * E, D), U2)
    return U1p, U2p


def _pack_core_weights(W1c, W2c, Wq, Wo, C6, M_res, G):
    nl = W1c.shape[0]
    U1p, U2p = _host_node_weights(
        np.asarray(W1c, np.float64), np.asarray(W2c, np.float64),
        np.asarray(Wq, np.float64), np.asarray(Wo, np.float64))
    # Block-diagonal mixing weights, one [128, 128] pair (U1bd, U2bd) per
    # (tile, quad, mode); node n sits at diagonal block (32g, 32g).
    bdw = np.zeros((128, NTILES * BDC), np.float32)
    for n in range(nl):
        jt, loc = divmod(n, NT)
        c, g = divmod(loc, 4)
        for m in range(3):
            base = jt * BDC + ((c * 3 + m) * 2) * 128
            sl = slice(32 * g, 32 * g + 32)
            bdw[sl, base + 32 * g:base + 32 * g + 32] = U1p[n, m]
            bdw[sl, base + 128 + 32 * g:base + 128 + 32 * g + 32] = U2p[n, m]
    # P-projection stationary, padded to 128 cols per batch for FWL:
    # block b = [96, 128] with C6 at cols 6b..6b+6.
    ccp = np.zeros((96, 8 * 128), np.float32)
    for b in range(8):
        ccp[:, 128 * b + 6 * b:128 * b + 6 * b + 6] = C6
    # G stationary per batch, padded to 128 cols: row 6b+cp carries G[:, cp]
    # (zt rows are 6b+cp after the b-major zq relayout).
    gtp = np.zeros((48, 8 * 128), np.float32)
    for b in range(8):
        for c in range(6):
            gtp[6 * b + c, 128 * b:128 * b + 96] = G[:, c]
    # M_res.T padded to [96, 128]
    matp = np.zeros((96, 128), np.float32)
    matp[:, :96] = M_res.T
    return {
        "bdw": bdw.astype(np.float16),
        "ccp": ccp.astype(NP_BF16),
        "gtp": gtp.astype(np.float16),
        "matp": matp.astype(NP_BF16),
        "id48": np.eye(48, dtype=np.float16),
        "id128": np.eye(128, dtype=np.float16),
    }


def shard_inputs(inputs):
    x = np.asarray(inputs["x"])
    W1, W2 = np.asarray(inputs["W1"]), np.asarray(inputs["W2"])
    Wq, Wo = np.asarray(inputs["Wq"]), np.asarray(inputs["Wo"])
    C6, M_res, G = _host_constants()
    # [B, N, L, D] -> [B, L, N, D]: per-(b,t) rows have nodes contiguous,
    # giving 1 KB bf16 DMA descriptors.
    xt = np.ascontiguousarray(x.transpose(0, 2, 1, 3)).astype(NP_BF16)
    maps = []
    for c in range(NCORES):
        n0 = c * NNODE
        xc = np.ascontiguousarray(
            xt[:, :, n0:n0 + NNODE]).reshape(NB * L, NNODE * D)
        m = {"xin": xc}
        m.update(_pack_core_weights(W1[n0:n0 + NNODE], W2[n0:n0 + NNODE],
                                    Wq, Wo, C6, M_res, G))
        maps.append(m)
    return maps


def unshard(rvals):
    """rvals: [NCORES, NB*L, NNODE*D] bf16 -> [NB, N, L, D] f32."""
    out = np.empty((NB, NCORES * NNODE, L, D), np.float32)
    for c in range(NCORES):
        rc = np.asarray(rvals[c]).astype(np.float32).reshape(NB, L, NNODE, D)
        out[:, c * NNODE:(c + 1) * NNODE] = rc.transpose(0, 2, 1, 3)
    return out


def _ap2(t, w, base, d0, n0, d1, n1):
    """2-level free AP [w, n0, n1] into tile t at column `base`."""
    a = t[:w, base:base + 1]
    return bass.AP(tensor=a.tensor, offset=a.offset,
                   ap=[list(a.ap[0]), [d0, n0], [d1, n1]])


def build_kernel(reps=1, pipe=True, mixmerge=True):
    nl = NNODE
    nc = bacc.Bacc("TRN2", target_bir_lowering=False, debug=False,
                   num_devices=NCORES)
    xin = nc.dram_tensor("xin", [NB * L, nl * D], BF16, kind="ExternalInput")
    bdw = nc.dram_tensor("bdw", [128, NTILES * BDC], F16, kind="ExternalInput")
    ccp = nc.dram_tensor("ccp", [96, 8 * 128], BF16, kind="ExternalInput")
    gtp = nc.dram_tensor("gtp", [48, 8 * 128], F16, kind="ExternalInput")
    matp = nc.dram_tensor("matp", [96, 128], BF16, kind="ExternalInput")
    id48 = nc.dram_tensor("id48", [48, 48], F16, kind="ExternalInput")
    id128 = nc.dram_tensor("id128", [128, 128], F16, kind="ExternalInput")
    rout = nc.dram_tensor("rout", [NB * L, nl * D], BF16, kind="ExternalOutput")

    work = [jt for _ in range(reps) for jt in range(NTILES)]
    n = len(work)

    with TileContext(nc) as tc:
        with (
            tc.tile_pool(name="consts", bufs=1) as consts,
            tc.tile_pool(name="xp", bufs=4) as xp,
            tc.tile_pool(name="bdp", bufs=3) as bdp,
            tc.tile_pool(name="psb", bufs=2) as psb,
            tc.tile_pool(name="ptsb", bufs=3) as ptsb,
            tc.tile_pool(name="zqsb", bufs=2) as zqsb,
            tc.tile_pool(name="ztsb", bufs=3) as ztsb,
            tc.tile_pool(name="outp", bufs=2) as outp,
            tc.tile_pool(name="ps48p", bufs=1, space="PSUM") as ps48p,
            tc.tile_pool(name="ptp", bufs=1, space="PSUM") as ptp,
            tc.tile_pool(name="zqp", bufs=1, space="PSUM") as zqp,
            tc.tile_pool(name="ztp", bufs=1, space="PSUM") as ztp,
            tc.tile_pool(name="resp", bufs=2, space="PSUM") as resp,
        ):
            cc_sb = consts.tile([96, 8 * 128], BF16)
            nc.sync.dma_start(out=cc_sb[:], in_=ccp.ap())
            gt_sb = consts.tile([48, 8 * 128], F16)
            nc.sync.dma_start(out=gt_sb[:], in_=gtp.ap())
            mat_sb = consts.tile([96, 128], BF16)
            nc.sync.dma_start(out=mat_sb[:], in_=matp.ap())
            id48_sb = consts.tile([48, 48], F16)
            nc.sync.dma_start(out=id48_sb[:], in_=id48.ap())
            id128_sb = consts.tile([128, 128], F16)
            nc.sync.dma_start(out=id128_sb[:], in_=id128.ap())

            st = {}     # round index -> per-item state dict
            pair = {}   # i//2 -> shared pair state (xt2/bdt2/out2)

            def s_load(i):
                """DMA x + bd-weights, one paired transfer per 2 tiles."""
                jt = work[i]
                nt = min(NT, nl - jt * NT)
                C = nt * 32
                if i % 2 == 0:
                    jt1 = work[i + 1] if i + 1 < n and work[i + 1] == jt + 1 \
                        else None
                    nt1 = min(NT, nl - (jt + 1) * NT) if jt1 is not None else 0
                    pitch = C + nt1 * 32
                    xt2 = xp.tile([96, NB * 1024], BF16, tag="xt2")
                    src = bass.AP(tensor=xin, offset=jt * NT * D,
                                  ap=[[nl * D, 96], [L * nl * D, NB],
                                      [1, pitch]])
                    nc.sync.dma_start(out=xt2[:, :NB * pitch], in_=src)
                    bdt2 = bdp.tile([128, 2 * BDC], F16, tag="bdt2")
                    nbd = 2 * BDC if jt1 is not None else BDC
                    nc.sync.dma_start(
                        out=bdt2[:, :nbd],
                        in_=bass.AP(tensor=bdw, offset=jt * BDC,
                                    ap=[[NTILES * BDC, 128], [1, nbd]]))
                    pair[i // 2] = {"xt2": xt2, "bdt2": bdt2, "pitch": pitch,
                                    "C0": C, "jt0": jt}
                p = pair[i // 2]
                off = 0 if i % 2 == 0 else p["C0"]
                return {"jt": jt, "nt": nt, "C": C, "p": p, "xoff": off,
                        "bdt": p["bdt2"], "bdoff": (i % 2) * BDC}

            def xtb(s, b):
                """Moving x slice for batch b of item s."""
                p = s["p"]
                base = b * p["pitch"] + s["xoff"]
                return p["xt2"][:, base:base + s["C"]]

            def s_mma(i):
                """P projection: 8 accumulating matmuls into ps48."""
                s = st[i]
                C = s["C"]
                ps48 = ps48p.tile([128, 512], F32, tag="ps48")
                for b in range(NB):
                    nc.tensor.matmul(
                        ps48[:, :C], cc_sb[:, 128 * b:128 * (b + 1)],
                        xtb(s, b), start=(b == 0), stop=(b == NB - 1))
                ps_sb = psb.tile([48, 512], F16, tag="ps_sb")
                nc.vector.tensor_copy(ps_sb[:, :C], ps48[:48, :C])
                s["ps_sb"] = ps_sb

            def s_transp(i):
                """PE-transpose P to [node*dim, (b, comp)] + negated mirror.

                pt_sb layout: region A [0,192): negated im comps at
                48c+m+6b; region B [192,384): transposed P at 192+48c+cp+6b.
                """
                s = st[i]
                C, nqt = s["C"], (s["nt"] + 3) // 4
                pt_ps = ptp.tile([128, 192], F16, tag="pt")
                pt_sb = ptsb.tile([128, 384], F16, tag="pt_sb")
                for c in range(nqt):
                    w = min(128, C - 128 * c)
                    nc.tensor.transpose(
                        pt_ps[:w, 48 * c:48 * c + 48],
                        s["ps_sb"][:, 128 * c:128 * c + w], id48_sb[:])
                    nc.scalar.copy(
                        out=pt_sb[:w, 192 + 48 * c:192 + 48 * c + 48],
                        in_=pt_ps[:w, 48 * c:48 * c + 48])
                    # negated im comps mirrored into region A (same 6-stride
                    # layout as region B so u2's moving AP spans both).
                    for m in range(3):
                        sH = 48 * c + 3 + m
                        nc.vector.tensor_scalar_mul(
                            pt_sb[:w, 48 * c + m:48 * c + m + 6 * (NB - 1) + 1:6],
                            pt_ps[:w, sH:sH + 6 * (NB - 1) + 1:6],
                            -1.0)
                s["pt_sb"] = pt_sb

            def s_mix(i):
                """Per-node complex mixing, 2 16-col matmuls per (quad, mode)."""
                s = st[i]
                nqt = (s["nt"] + 3) // 4
                bdt, bd0 = s["bdt"], s["bdoff"]
                pt_sb = s["pt_sb"]
                zq_ps = zqp.tile([128, 192], F32, tag="zq")
                for c in range(nqt):
                    for m in range(3):
                        u1 = bdt[:, bd0 + ((c * 3 + m) * 2) * 128:
                                 bd0 + ((c * 3 + m) * 2) * 128 + 128]
                        u2 = bdt[:, bd0 + ((c * 3 + m) * 2 + 1) * 128:
                                 bd0 + ((c * 3 + m) * 2 + 1) * 128 + 128]
                        if mixmerge:
                            # b-major zq: col 48c+6b+cp.  out order (j, b):
                            # j=0 -> re (cp=m), j=1 -> im (cp=3+m)
                            zout = _ap2(zq_ps, 128, 48 * c + m, 3, 2, 6, 8)
                            # u1 moving: j=0 Pre(m), j=1 Pim(m) (region B)
                            m1 = _ap2(pt_sb, 128, 192 + 48 * c + m, 3, 2, 6, 8)
                            # u2 moving: j=0 -Pim(m) (region A), j=1 Pre(m)
                            m2 = _ap2(pt_sb, 128, 48 * c + m, 192, 2, 6, 8)
                            nc.tensor.matmul(zout, u1, m1, start=True,
                                             stop=False)
                            nc.tensor.matmul(zout, u2, m2, start=False,
                                             stop=True)
                        else:
                            def pcols(cidx):
                                sH = 192 + 48 * c + cidx
                                return pt_sb[:, sH:sH + 6 * (NB - 1) + 1:6]

                            def zcols(cp):
                                sH = 48 * c + cp
                                return zq_ps[:, sH:sH + 6 * (NB - 1) + 1:6]

                            pn = pt_sb[:, 48 * c + m:
                                       48 * c + m + 6 * (NB - 1) + 1:6]
                            nc.tensor.matmul(zcols(m), u1, pcols(m),
                                             start=True, stop=False)
                            nc.tensor.matmul(zcols(3 + m), u1, pcols(3 + m),
                                             start=True, stop=False)
                            nc.tensor.matmul(zcols(m), u2, pn,
                                             start=False, stop=True)
                            nc.tensor.matmul(zcols(3 + m), u2, pcols(m),
                                             start=False, stop=True)
                zq_sb = zqsb.tile([128, 192], F16, tag="zq_sb")
                nc.vector.tensor_copy(zq_sb[:, :48 * nqt], zq_ps[:, :48 * nqt])
                s["zq_sb"] = zq_sb

            def s_ztt(i):
                """Transpose Z back to [(b, comp), (node, dim)]."""
                s = st[i]
                C, nqt = s["C"], (s["nt"] + 3) // 4
                zt_ps = ztp.tile([48, 512], F16, tag="zt")
                zt_sb = ztsb.tile([48, 512], F16, tag="zt_sb")
                for c in range(nqt):
                    w = min(128, C - 128 * c)
                    nc.tensor.transpose(
                        zt_ps[:, 128 * c:128 * c + w],
                        s["zq_sb"][:w, 48 * c:48 * c + 48], id128_sb[:w, :w])
                    nc.vector.tensor_copy(
                        zt_sb[:, 128 * c:128 * c + w],
                        zt_ps[:, 128 * c:128 * c + w])
                s["zt_sb"] = zt_sb

            def s_b(i):
                """Residual: res = M_res @ x + G @ Z, evac, store.

                Paired so the shared matp stationary is loaded once per pair;
                output DMA covers 2 tiles (s_b of the odd item issues it).
                """
                s = st[i]
                C, pr = s["C"], s["p"]
                pitch = pr["pitch"]
                if i % 2 == 0:
                    out2 = outp.tile([96, NB * 1024], BF16, tag="out2")
                    pr["out2"] = out2
                else:
                    out2 = pr["out2"]
                ooff = s["xoff"]
                for p in range(NB // 2):
                    b0, b1 = 2 * p, 2 * p + 1
                    r0 = resp.tile([128, 512], F32, tag="res0")
                    r1 = resp.tile([128, 512], F32, tag="res1")
                    nc.tensor.matmul(r0[:, :C], mat_sb[:], xtb(s, b0),
                                     start=True, stop=False)
                    nc.tensor.matmul(r1[:, :C], mat_sb[:], xtb(s, b1),
                                     start=True, stop=False)
                    nc.tensor.matmul(r0[:, :C],
                                     gt_sb[:, 128 * b0:128 * (b0 + 1)],
                                     s["zt_sb"][:, :C], start=False, stop=True)
                    nc.tensor.matmul(r1[:, :C],
                                     gt_sb[:, 128 * b1:128 * (b1 + 1)],
                                     s["zt_sb"][:, :C], start=False, stop=True)
                    nc.vector.tensor_copy(
                        out2[:, b0 * pitch + ooff:b0 * pitch + ooff + C],
                        r0[:96, :C])
                    nc.scalar.copy(
                        out=out2[:, b1 * pitch + ooff:b1 * pitch + ooff + C],
                        in_=r1[:96, :C])
                last = (i % 2 == 1) or (i + 1 >= n) or (work[i + 1] != s["jt"] + 1)
                if last:
                    dst = bass.AP(tensor=rout, offset=pr["jt0"] * NT * D,
                                  ap=[[nl * D, 96], [L * nl * D, NB],
                                      [1, pitch]])
                    nc.sync.dma_start(out=dst, in_=out2[:, :NB * pitch])
                    pair.pop(i // 2, None)

            if pipe:
                # software pipeline: load(i+2) | mma(i) | transp(i-1)
                #                    | mix(i-2) | b(i-3) | ztt(i-2)
                st[0] = s_load(0)
                st[1] = s_load(1)
                for i in range(n + 3):
                    if i + 2 < n:
                        st[i + 2] = s_load(i + 2)
                    if i < n:
                        s_mma(i)
                    if 0 <= i - 1 < n:
                        s_transp(i - 1)
                    if 0 <= i - 2 < n:
                        s_mix(i - 2)
                    if 0 <= i - 3 < n:
                        s_b(i - 3)
                        st.pop(i - 3)
                    if 0 <= i - 2 < n:
                        s_ztt(i - 2)
            else:
                st[0] = s_load(0)
                for i in range(n):
                    if i + 1 < n:
                        st[i + 1] = s_load(i + 1)
                    s_mma(i)
                    s_transp(i)
                    s_mix(i)
                    s_ztt(i)
                    s_b(i)
                    st.pop(i)
    nc.compile()
    return nc


from concourse.bass_utils import run_bass_kernel_spmd

_NC_CACHE = None


def kernel(x, Wq, bq, Wk, bk, Wv, bv, Wo, bo, W1, W2):
    """Full inputs -> full output res [8, 2000, 96, 32] float32."""
    global _NC_CACHE
    maps = shard_inputs({"x": np.asarray(x), "W1": np.asarray(W1),
                         "W2": np.asarray(W2), "Wq": np.asarray(Wq),
                         "Wo": np.asarray(Wo)})
    if _NC_CACHE is None:
        _NC_CACHE = build_kernel(reps=1)
    res = run_bass_kernel_spmd(_NC_CACHE, maps, list(range(NCORES)))
    rvals = [res.results[c]["rout"] for c in range(NCORES)]
    return unshard(rvals)


# revision 60
# speedup vs baseline: 1.0372x; 1.0372x over previous
"""Trainium2 Bass kernel for nn_FELDMSTM_7988639171122 (8 NeuronCores).

v5: v4 + FWL-friendly 128-col stationaries (fast weight load), s_b
stationary-reuse ordering, and merged 16-col mixing matmuls.

Math per (b, n) sample x_s [L=96, D=32]:
  P_s = C6.T @ x_s  [6, 32];  Z = per-node complex mixing of P via U1p/U2p;
  res = M_res @ x_s + G @ Z.
"""

import numpy as np
import ml_dtypes
import concourse.bass as bass
import concourse.bacc as bacc
import concourse.mybir as mybir
from concourse.tile import TileContext

F32 = mybir.dt.float32
F16 = mybir.dt.float16
BF16 = mybir.dt.bfloat16
NP_BF16 = ml_dtypes.bfloat16

L, D, H, E, MODES, KAVG = 96, 32, 4, 8, (1, 4, 5), 25
NB = 8          # batch
NNODE = 250     # nodes per core
NCORES = 8
NT = 16         # nodes per device tile
TD = L * D      # 3072
NTILES = (NNODE + NT - 1) // NT
BDC = 24 * 128  # bd-weight cols per tile: 4 quads x 3 modes x 2 mats x 128


def _host_constants():
    t = np.arange(L)
    th = 2 * np.pi * np.outer(t, np.array(MODES)) / L
    C6 = np.concatenate([np.cos(th), -np.sin(th)], axis=1)
    pad = (KAVG - 1) // 2
    A = np.zeros((L, L))
    for tt in range(L):
        for w in range(KAVG):
            A[tt, min(max(tt + w - pad, 0), L - 1)] += 1.0 / KAVG
    M_res = np.eye(L) - A
    Cinv = np.concatenate([(2.0 / L) * np.cos(th), -(2.0 / L) * np.sin(th)], axis=1)
    G = M_res @ Cinv
    return C6, M_res, G


def _host_node_weights(W1, W2, Wq, Wo):
    N = W1.shape[0]
    WoT = Wo.T.reshape(H, E, D)
    U1 = np.einsum("nheom,hod->nmhed", W1, WoT).reshape(N, 3, H * E, D)
    U2 = np.einsum("nheom,hod->nmhed", W2, WoT).reshape(N, 3, H * E, D)
    U1p = np.einsum("hd,nmhe->nmde", Wq.reshape(H * E, D), U1)
    U2p = np.einsum("hd,nmhe->nmde", Wq.reshape(H * E, D), U2)
    return U1p, U2p


def _pack_core_weights(W1c, W2c, Wq, Wo, C6, M_res, G):
    nl = W1c.shape[0]
    U1p, U2p = _host_node_weights(
        np.asarray(W1c, np.float64), np.asarray(W2c, np.float64),
        np.asarray(Wq, np.float64), np.asarray(Wo, np.float64))
    # Block-diagonal mixing weights, one [128, 128] pair (U1bd, U2bd) per
    # (tile, quad, mode); node n sits at diagonal block (32g, 32g).
    bdw = np.zeros((128, NTILES * BDC), np.float32)
    for n in range(nl):
        jt, loc = divmod(n, NT)
        c, g = divmod(loc, 4)
        for m in range(3):
            base = jt * BDC + ((c * 3 + m) * 2) * 128
            sl = slice(32 * g, 32 * g + 32)
            bdw[sl, base + 32 * g:base + 32 * g + 32] = U1p[n, m]
            bdw[sl, base + 128 + 32 * g:base + 128 + 32 * g + 32] = U2p[n, m]
    # P-projection stationary, padded to 128 cols per batch for FWL:
    # block b = [96, 128] with C6 at cols 6b..6b+6.
    ccp = np.zeros((96, 8 * 128), np.float32)
    for b in range(8):
        ccp[:, 128 * b + 6 * b:128 * b + 6 * b + 6] = C6
    # G stationary per batch, padded to 128 cols: row 6b+cp carries G[:, cp]
    # (zt rows are 6b+cp after the b-major zq relayout).
    gtp = np.zeros((48, 8 * 128), np.float32)
    for b in range(8):
        for c in range(6):
            gtp[6 * b + c, 128 * b:128 * b + 96] = G[:, c]
    # M_res.T padded to [96, 128]
    matp = np.zeros((96, 128), np.float32)
    matp[:, :96] = M_res.T
    return {
        "bdw": bdw.astype(np.float16),
        "ccp": ccp.astype(NP_BF16),
        "gtp": gtp.astype(np.float16),
        "matp": matp.astype(NP_BF16),
        "id48": np.eye(48, dtype=np.float16),
        "id128": np.eye(128, dtype=np.float16),
    }


def shard_inputs(inputs):
    x = np.asarray(inputs["x"])
    W1, W2 = np.asarray(inputs["W1"]), np.asarray(inputs["W2"])
    Wq, Wo = np.asarray(inputs["Wq"]), np.asarray(inputs["Wo"])
    C6, M_res, G = _host_constants()
    # [B, N, L, D] -> [B, L, N, D]: per-(b,t) rows have nodes contiguous,
    # giving 1 KB bf16 DMA descriptors.
    xt = np.ascontiguousarray(x.transpose(0, 2, 1, 3)).astype(NP_BF16)
    maps = []
    for c in range(NCORES):
        n0 = c * NNODE
        xc = np.ascontiguousarray(
            xt[:, :, n0:n0 + NNODE]).reshape(NB * L, NNODE * D)
        m = {"xin": xc}
        m.update(_pack_core_weights(W1[n0:n0 + NNODE], W2[n0:n0 + NNODE],
                                    Wq, Wo, C6, M_res, G))
        maps.append(m)
    return maps


def unshard(rvals):
    """rvals: [NCORES, NB*L, NNODE*D] bf16 -> [NB, N, L, D] f32."""
    out = np.empty((NB, NCORES * NNODE, L, D), np.float32)
    for c in range(NCORES):
        rc = np.asarray(rvals[c]).astype(np.float32).reshape(NB, L, NNODE, D)
        out[:, c * NNODE:(c + 1) * NNODE] = rc.transpose(0, 2, 1, 3)
    return out


def _ap2(t, w, base, d0, n0, d1, n1):
    """2-level free AP [w, n0, n1] into tile t at column `base`."""
    a = t[:w, base:base + 1]
    return bass.AP(tensor=a.tensor, offset=a.offset,
                   ap=[list(a.ap[0]), [d0, n0], [d1, n1]])


def build_kernel(reps=1, pipe=True, mixmerge=True):
    nl = NNODE
    nc = bacc.Bacc("TRN2", target_bir_lowering=False, debug=False,
                   num_devices=NCORES)
    xin = nc.dram_tensor("xin", [NB * L, nl * D], BF16, kind="ExternalInput")
    bdw = nc.dram_tensor("bdw", [128, NTILES * BDC], F16, kind="ExternalInput")
    ccp = nc.dram_tensor("ccp", [96, 8 * 128], BF16, kind="ExternalInput")
    gtp = nc.dram_tensor("gtp", [48, 8 * 128], F16, kind="ExternalInput")
    matp = nc.dram_tensor("matp", [96, 128], BF16, kind="ExternalInput")
    id48 = nc.dram_tensor("id48", [48, 48], F16, kind="ExternalInput")
    id128 = nc.dram_tensor("id128", [128, 128], F16, kind="ExternalInput")
    rout = nc.dram_tensor("rout", [NB * L, nl * D], BF16, kind="ExternalOutput")

    work = [jt for _ in range(reps) for jt in range(NTILES)]
    n = len(work)

    with TileContext(nc) as tc:
        with (
            tc.tile_pool(name="consts", bufs=1) as consts,
            tc.tile_pool(name="xp", bufs=4) as xp,
            tc.tile_pool(name="bdp", bufs=3) as bdp,
            tc.tile_pool(name="psb", bufs=2) as psb,
            tc.tile_pool(name="ptsb", bufs=3) as ptsb,
            tc.tile_pool(name="zqsb", bufs=2) as zqsb,
            tc.tile_pool(name="ztsb", bufs=3) as ztsb,
            tc.tile_pool(name="outp", bufs=2) as outp,
            tc.tile_pool(name="ps48p", bufs=1, space="PSUM") as ps48p,
            tc.tile_pool(name="ptp", bufs=1, space="PSUM") as ptp,
            tc.tile_pool(name="zqp", bufs=1, space="PSUM") as zqp,
            tc.tile_pool(name="ztp", bufs=1, space="PSUM") as ztp,
            tc.tile_pool(name="resp", bufs=2, space="PSUM") as resp,
        ):
            cc_sb = consts.tile([96, 8 * 128], BF16)
            nc.sync.dma_start(out=cc_sb[:], in_=ccp.ap())
            gt_sb = consts.tile([48, 8 * 128], F16)
            nc.sync.dma_start(out=gt_sb[:], in_=gtp.ap())
            mat_sb = consts.tile([96, 128], BF16)
            nc.sync.dma_start(out=mat_sb[:], in_=matp.ap())
            id48_sb = consts.tile([48, 48], F16)
            nc.sync.dma_start(out=id48_sb[:], in_=id48.ap())
            id128_sb = consts.tile([128, 128], F16)
            nc.sync.dma_start(out=id128_sb[:], in_=id128.ap())

            st = {}     # round index -> per-item state dict
            pair = {}   # i//2 -> shared pair state (xt2/bdt2/out2)

            def s_load(i):
                """DMA x + bd-weights, one paired transfer per 2 tiles."""
                jt = work[i]
                nt = min(NT, nl - jt * NT)
                C = nt * 32
                if i % 2 == 0:
                    jt1 = work[i + 1] if i + 1 < n and work[i + 1] == jt + 1 \
                        else None
                    nt1 = min(NT, nl - (jt + 1) * NT) if jt1 is not None else 0
                    pitch = C + nt1 * 32
                    xt2 = xp.tile([96, NB * 1024], BF16, tag="xt2")
                    src = bass.AP(tensor=xin, offset=jt * NT * D,
                                  ap=[[nl * D, 96], [L * nl * D, NB],
                                      [1, pitch]])
                    nc.sync.dma_start(out=xt2[:, :NB * pitch], in_=src)
                    bdt2 = bdp.tile([128, 2 * BDC], F16, tag="bdt2")
                    nbd = 2 * BDC if jt1 is not None else BDC
                    nc.sync.dma_start(
                        out=bdt2[:, :nbd],
                        in_=bass.AP(tensor=bdw, offset=jt * BDC,
                                    ap=[[NTILES * BDC, 128], [1, nbd]]))
                    pair[i // 2] = {"xt2": xt2, "bdt2": bdt2, "pitch": pitch,
                                    "C0": C, "jt0": jt}
                p = pair[i // 2]
                off = 0 if i % 2 == 0 else p["C0"]
                return {"jt": jt, "nt": nt, "C": C, "p": p, "xoff": off,
                        "bdt": p["bdt2"], "bdoff": (i % 2) * BDC}

            def xtb(s, b):
                """Moving x slice for batch b of item s."""
                p = s["p"]
                base = b * p["pitch"] + s["xoff"]
                return p["xt2"][:, base:base + s["C"]]

            def s_mma(i):
                """P projection: 8 accumulating matmuls into ps48."""
                s = st[i]
                C = s["C"]
                ps48 = ps48p.tile([128, 512], F32, tag="ps48")
                for b in range(NB):
                    nc.tensor.matmul(
                        ps48[:, :C], cc_sb[:, 128 * b:128 * (b + 1)],
                        xtb(s, b), start=(b == 0), stop=(b == NB - 1))
                ps_sb = psb.tile([48, 512], F16, tag="ps_sb")
                nc.vector.tensor_copy(ps_sb[:, :C], ps48[:48, :C])
                s["ps_sb"] = ps_sb

            def s_transp(i):
                """PE-transpose P to [node*dim, (b, comp)] + negated mirror.

                pt_sb layout: region A [0,192): negated im comps at
                48c+m+6b; region B [192,384): transposed P at 192+48c+cp+6b.
                """
                s = st[i]
                C, nqt = s["C"], (s["nt"] + 3) // 4
                pt_ps = ptp.tile([128, 192], F16, tag="pt")
                pt_sb = ptsb.tile([128, 384], F16, tag="pt_sb")
                for c in range(nqt):
                    w = min(128, C - 128 * c)
                    nc.tensor.transpose(
                        pt_ps[:w, 48 * c:48 * c + 48],
                        s["ps_sb"][:, 128 * c:128 * c + w], id48_sb[:])
                    nc.vector.tensor_copy(
                        pt_sb[:w, 192 + 48 * c:192 + 48 * c + 48],
                        pt_ps[:w, 48 * c:48 * c + 48])
                    # negated im comps mirrored into region A (same 6-stride
                    # layout as region B so u2's moving AP spans both).
                    # Sourced from the region-B copy (SBUF) so the idle
                    # GpSimd engine can run them off the DVE queue.
                    for m in range(3):
                        sB = 192 + 48 * c + 3 + m
                        nc.gpsimd.tensor_scalar_mul(
                            pt_sb[:w, 48 * c + m:48 * c + m + 6 * (NB - 1) + 1:6],
                            pt_sb[:w, sB:sB + 6 * (NB - 1) + 1:6],
                            -1.0)
                s["pt_sb"] = pt_sb

            def s_mix(i):
                """Per-node complex mixing, 2 16-col matmuls per (quad, mode)."""
                s = st[i]
                nqt = (s["nt"] + 3) // 4
                bdt, bd0 = s["bdt"], s["bdoff"]
                pt_sb = s["pt_sb"]
                zq_ps = zqp.tile([128, 192], F32, tag="zq")
                for c in range(nqt):
                    for m in range(3):
                        u1 = bdt[:, bd0 + ((c * 3 + m) * 2) * 128:
                                 bd0 + ((c * 3 + m) * 2) * 128 + 128]
                        u2 = bdt[:, bd0 + ((c * 3 + m) * 2 + 1) * 128:
                                 bd0 + ((c * 3 + m) * 2 + 1) * 128 + 128]
                        if mixmerge:
                            # b-major zq: col 48c+6b+cp.  out order (j, b):
                            # j=0 -> re (cp=m), j=1 -> im (cp=3+m)
                            zout = _ap2(zq_ps, 128, 48 * c + m, 3, 2, 6, 8)
                            # u1 moving: j=0 Pre(m), j=1 Pim(m) (region B)
                            m1 = _ap2(pt_sb, 128, 192 + 48 * c + m, 3, 2, 6, 8)
                            # u2 moving: j=0 -Pim(m) (region A), j=1 Pre(m)
                            m2 = _ap2(pt_sb, 128, 48 * c + m, 192, 2, 6, 8)
                            nc.tensor.matmul(zout, u1, m1, start=True,
                                             stop=False)
                            nc.tensor.matmul(zout, u2, m2, start=False,
                                             stop=True)
                        else:
                            def pcols(cidx):
                                sH = 192 + 48 * c + cidx
                                return pt_sb[:, sH:sH + 6 * (NB - 1) + 1:6]

                            def zcols(cp):
                                sH = 48 * c + cp
                                return zq_ps[:, sH:sH + 6 * (NB - 1) + 1:6]

                            pn = pt_sb[:, 48 * c + m:
                                       48 * c + m + 6 * (NB - 1) + 1:6]
                            nc.tensor.matmul(zcols(m), u1, pcols(m),
                                             start=True, stop=False)
                            nc.tensor.matmul(zcols(3 + m), u1, pcols(3 + m),
                                             start=True, stop=False)
                            nc.tensor.matmul(zcols(m), u2, pn,
                                             start=False, stop=True)
                            nc.tensor.matmul(zcols(3 + m), u2, pcols(m),
                                             start=False, stop=True)
                zq_sb = zqsb.tile([128, 192], F16, tag="zq_sb")
                nc.vector.tensor_copy(zq_sb[:, :48 * nqt], zq_ps[:, :48 * nqt])
                s["zq_sb"] = zq_sb

            def s_ztt(i):
                """Transpose Z back to [(b, comp), (node, dim)]."""
                s = st[i]
                C, nqt = s["C"], (s["nt"] + 3) // 4
                zt_ps = ztp.tile([48, 512], F16, tag="zt")
                zt_sb = ztsb.tile([48, 512], F16, tag="zt_sb")
                for c in range(nqt):
                    w = min(128, C - 128 * c)
                    nc.tensor.transpose(
                        zt_ps[:, 128 * c:128 * c + w],
                        s["zq_sb"][:w, 48 * c:48 * c + 48], id128_sb[:w, :w])
                    nc.vector.tensor_copy(
                        zt_sb[:, 128 * c:128 * c + w],
                        zt_ps[:, 128 * c:128 * c + w])
                s["zt_sb"] = zt_sb

            def s_b(i):
                """Residual: res = M_res @ x + G @ Z, evac, store.

                Paired so the shared matp stationary is loaded once per pair;
                output DMA covers 2 tiles (s_b of the odd item issues it).
                """
                s = st[i]
                C, pr = s["C"], s["p"]
                pitch = pr["pitch"]
                if i % 2 == 0:
                    out2 = outp.tile([96, NB * 1024], BF16, tag="out2")
                    pr["out2"] = out2
                else:
                    out2 = pr["out2"]
                ooff = s["xoff"]
                for p in range(NB // 2):
                    b0, b1 = 2 * p, 2 * p + 1
                    r0 = resp.tile([128, 512], F32, tag="res0")
                    r1 = resp.tile([128, 512], F32, tag="res1")
                    nc.tensor.matmul(r0[:, :C], mat_sb[:], xtb(s, b0),
                                     start=True, stop=False)
                    nc.tensor.matmul(r1[:, :C], mat_sb[:], xtb(s, b1),
                                     start=True, stop=False)
                    nc.tensor.matmul(r0[:, :C],
                                     gt_sb[:, 128 * b0:128 * (b0 + 1)],
                                     s["zt_sb"][:, :C], start=False, stop=True)
                    nc.tensor.matmul(r1[:, :C],
                                     gt_sb[:, 128 * b1:128 * (b1 + 1)],
                                     s["zt_sb"][:, :C], start=False, stop=True)
                    nc.vector.tensor_copy(
                        out2[:, b0 * pitch + ooff:b0 * pitch + ooff + C],
                        r0[:96, :C])
                    nc.scalar.copy(
                        out=out2[:, b1 * pitch + ooff:b1 * pitch + ooff + C],
                        in_=r1[:96, :C])
                last = (i % 2 == 1) or (i + 1 >= n) or (work[i + 1] != s["jt"] + 1)
                if last:
                    dst = bass.AP(tensor=rout, offset=pr["jt0"] * NT * D,
                                  ap=[[nl * D, 96], [L * nl * D, NB],
                                      [1, pitch]])
                    nc.sync.dma_start(out=dst, in_=out2[:, :NB * pitch])
                    pair.pop(i // 2, None)

            if pipe:
                # software pipeline: load(i+2) | mma(i) | transp(i-1)
                #                    | mix(i-2) | b(i-3) | ztt(i-2)
                st[0] = s_load(0)
                st[1] = s_load(1)
                for i in range(n + 3):
                    if i + 2 < n:
                        st[i + 2] = s_load(i + 2)
                    if i < n:
                        s_mma(i)
                    if 0 <= i - 1 < n:
                        s_transp(i - 1)
                    if 0 <= i - 2 < n:
                        s_mix(i - 2)
                    if 0 <= i - 3 < n:
                        s_b(i - 3)
                        st.pop(i - 3)
                    if 0 <= i - 2 < n:
                        s_ztt(i - 2)
            else:
                st[0] = s_load(0)
                for i in range(n):
                    if i + 1 < n:
                        st[i + 1] = s_load(i + 1)
                    s_mma(i)
                    s_transp(i)
                    s_mix(i)
                    s_ztt(i)
                    s_b(i)
                    st.pop(i)
    nc.compile()
    return nc


from concourse.bass_utils import run_bass_kernel_spmd

_NC_CACHE = None


def kernel(x, Wq, bq, Wk, bk, Wv, bv, Wo, bo, W1, W2):
    """Full inputs -> full output res [8, 2000, 96, 32] float32."""
    global _NC_CACHE
    maps = shard_inputs({"x": np.asarray(x), "W1": np.asarray(W1),
                         "W2": np.asarray(W2), "Wq": np.asarray(Wq),
                         "Wo": np.asarray(Wo)})
    if _NC_CACHE is None:
        _NC_CACHE = build_kernel(reps=1)
    res = run_bass_kernel_spmd(_NC_CACHE, maps, list(range(NCORES)))
    rvals = [res.results[c]["rout"] for c in range(NCORES)]
    return unshard(rvals)


# revision 62
# speedup vs baseline: 1.1525x; 1.1111x over previous
"""Trainium2 Bass kernel for nn_FELDMSTM_7988639171122 (8 NeuronCores).

v5: v4 + FWL-friendly 128-col stationaries (fast weight load), s_b
stationary-reuse ordering, and merged 16-col mixing matmuls.

Math per (b, n) sample x_s [L=96, D=32]:
  P_s = C6.T @ x_s  [6, 32];  Z = per-node complex mixing of P via U1p/U2p;
  res = M_res @ x_s + G @ Z.
"""

import numpy as np
import ml_dtypes
import concourse.bass as bass
import concourse.bacc as bacc
import concourse.mybir as mybir
from concourse.tile import TileContext

F32 = mybir.dt.float32
F16 = mybir.dt.float16
BF16 = mybir.dt.bfloat16
NP_BF16 = ml_dtypes.bfloat16

L, D, H, E, MODES, KAVG = 96, 32, 4, 8, (1, 4, 5), 25
NB = 8          # batch
NNODE = 250     # nodes per core
NCORES = 8
NT = 16         # nodes per device tile
TD = L * D      # 3072
NTILES = (NNODE + NT - 1) // NT
BDC = 24 * 128  # bd-weight cols per tile: 4 quads x 3 modes x 2 mats x 128


def _host_constants():
    t = np.arange(L)
    th = 2 * np.pi * np.outer(t, np.array(MODES)) / L
    C6 = np.concatenate([np.cos(th), -np.sin(th)], axis=1)
    pad = (KAVG - 1) // 2
    A = np.zeros((L, L))
    for tt in range(L):
        for w in range(KAVG):
            A[tt, min(max(tt + w - pad, 0), L - 1)] += 1.0 / KAVG
    M_res = np.eye(L) - A
    Cinv = np.concatenate([(2.0 / L) * np.cos(th), -(2.0 / L) * np.sin(th)], axis=1)
    G = M_res @ Cinv
    return C6, M_res, G


def _host_node_weights(W1, W2, Wq, Wo):
    N = W1.shape[0]
    WoT = Wo.T.reshape(H, E, D)
    U1 = np.einsum("nheom,hod->nmhed", W1, WoT).reshape(N, 3, H * E, D)
    U2 = np.einsum("nheom,hod->nmhed", W2, WoT).reshape(N, 3, H * E, D)
    U1p = np.einsum("hd,nmhe->nmde", Wq.reshape(H * E, D), U1)
    U2p = np.einsum("hd,nmhe->nmde", Wq.reshape(H * E, D), U2)
    return U1p, U2p


def _pack_core_weights(W1c, W2c, Wq, Wo, C6, M_res, G):
    nl = W1c.shape[0]
    U1p, U2p = _host_node_weights(
        np.asarray(W1c, np.float64), np.asarray(W2c, np.float64),
        np.asarray(Wq, np.float64), np.asarray(Wo, np.float64))
    # Block-diagonal mixing weights, one [128, 128] pair (U1bd, U2bd) per
    # (tile, quad, mode); node n sits at diagonal block (32g, 32g).
    bdw = np.zeros((128, NTILES * BDC), np.float32)
    for n in range(nl):
        jt, loc = divmod(n, NT)
        c, g = divmod(loc, 4)
        for m in range(3):
            base = jt * BDC + ((c * 3 + m) * 2) * 128
            sl = slice(32 * g, 32 * g + 32)
            bdw[sl, base + 32 * g:base + 32 * g + 32] = U1p[n, m]
            bdw[sl, base + 128 + 32 * g:base + 128 + 32 * g + 32] = U2p[n, m]
    # P-projection stationary, padded to 128 cols per batch for FWL:
    # block b = [96, 128] with C6 at cols 6b..6b+6.
    ccp = np.zeros((96, 8 * 128), np.float32)
    for b in range(8):
        ccp[:, 128 * b + 6 * b:128 * b + 6 * b + 6] = C6
    # G stationary per batch, padded to 128 cols: row 6b+cp carries G[:, cp]
    # (zt rows are 6b+cp after the b-major zq relayout).
    gtp = np.zeros((48, 8 * 128), np.float32)
    for b in range(8):
        for c in range(6):
            gtp[6 * b + c, 128 * b:128 * b + 96] = G[:, c]
    # M_res.T padded to [96, 128]
    matp = np.zeros((96, 128), np.float32)
    matp[:, :96] = M_res.T
    return {
        "bdw": bdw.astype(np.float16),
        "ccp": ccp.astype(NP_BF16),
        "gtp": gtp.astype(np.float16),
        "matp": matp.astype(NP_BF16),
        "id48": np.eye(48, dtype=np.float16),
        "id128": np.eye(128, dtype=np.float16),
    }


def shard_inputs(inputs):
    x = np.asarray(inputs["x"])
    W1, W2 = np.asarray(inputs["W1"]), np.asarray(inputs["W2"])
    Wq, Wo = np.asarray(inputs["Wq"]), np.asarray(inputs["Wo"])
    C6, M_res, G = _host_constants()
    # [B, N, L, D] -> [B, L, N, D]: per-(b,t) rows have nodes contiguous,
    # giving 1 KB bf16 DMA descriptors.
    xt = np.ascontiguousarray(x.transpose(0, 2, 1, 3)).astype(NP_BF16)
    maps = []
    for c in range(NCORES):
        n0 = c * NNODE
        xc = np.ascontiguousarray(
            xt[:, :, n0:n0 + NNODE]).reshape(NB * L, NNODE * D)
        m = {"xin": xc}
        m.update(_pack_core_weights(W1[n0:n0 + NNODE], W2[n0:n0 + NNODE],
                                    Wq, Wo, C6, M_res, G))
        maps.append(m)
    return maps


def unshard(rvals):
    """rvals: [NCORES, NB*L, NNODE*D] bf16 -> [NB, N, L, D] f32."""
    out = np.empty((NB, NCORES * NNODE, L, D), np.float32)
    for c in range(NCORES):
        rc = np.asarray(rvals[c]).astype(np.float32).reshape(NB, L, NNODE, D)
        out[:, c * NNODE:(c + 1) * NNODE] = rc.transpose(0, 2, 1, 3)
    return out


def _ap2(t, w, base, d0, n0, d1, n1):
    """2-level free AP [w, n0, n1] into tile t at column `base`."""
    a = t[:w, base:base + 1]
    return bass.AP(tensor=a.tensor, offset=a.offset,
                   ap=[list(a.ap[0]), [d0, n0], [d1, n1]])


def build_kernel(reps=1, pipe=True, mixmerge=True):
    nl = NNODE
    nc = bacc.Bacc("TRN2", target_bir_lowering=False, debug=False,
                   num_devices=NCORES)
    xin = nc.dram_tensor("xin", [NB * L, nl * D], BF16, kind="ExternalInput")
    bdw = nc.dram_tensor("bdw", [128, NTILES * BDC], F16, kind="ExternalInput")
    ccp = nc.dram_tensor("ccp", [96, 8 * 128], BF16, kind="ExternalInput")
    gtp = nc.dram_tensor("gtp", [48, 8 * 128], F16, kind="ExternalInput")
    matp = nc.dram_tensor("matp", [96, 128], BF16, kind="ExternalInput")
    id48 = nc.dram_tensor("id48", [48, 48], F16, kind="ExternalInput")
    id128 = nc.dram_tensor("id128", [128, 128], F16, kind="ExternalInput")
    rout = nc.dram_tensor("rout", [NB * L, nl * D], BF16, kind="ExternalOutput")

    work = [jt for _ in range(reps) for jt in range(NTILES)]
    n = len(work)

    with TileContext(nc) as tc:
        with (
            tc.tile_pool(name="consts", bufs=1) as consts,
            tc.tile_pool(name="xp", bufs=4) as xp,
            tc.tile_pool(name="bdp", bufs=3) as bdp,
            tc.tile_pool(name="psb", bufs=3) as psb,
            tc.tile_pool(name="ptsb", bufs=3) as ptsb,
            tc.tile_pool(name="zqsb", bufs=2) as zqsb,
            tc.tile_pool(name="ztsb", bufs=3) as ztsb,
            tc.tile_pool(name="outp", bufs=2) as outp,
            tc.tile_pool(name="ps48p", bufs=1, space="PSUM") as ps48p,
            tc.tile_pool(name="ptp", bufs=1, space="PSUM") as ptp,
            tc.tile_pool(name="zqp", bufs=1, space="PSUM") as zqp,
            tc.tile_pool(name="ztp", bufs=1, space="PSUM") as ztp,
            tc.tile_pool(name="resp", bufs=2, space="PSUM") as resp,
        ):
            cc_sb = consts.tile([96, 8 * 128], BF16)
            nc.sync.dma_start(out=cc_sb[:], in_=ccp.ap())
            gt_sb = consts.tile([48, 8 * 128], F16)
            nc.sync.dma_start(out=gt_sb[:], in_=gtp.ap())
            mat_sb = consts.tile([96, 128], BF16)
            nc.sync.dma_start(out=mat_sb[:], in_=matp.ap())
            id48_sb = consts.tile([48, 48], F16)
            nc.sync.dma_start(out=id48_sb[:], in_=id48.ap())
            id128_sb = consts.tile([128, 128], F16)
            nc.sync.dma_start(out=id128_sb[:], in_=id128.ap())

            st = {}     # round index -> per-item state dict
            pair = {}   # i//2 -> shared pair state (xt2/bdt2/out2)

            def s_load(i):
                """DMA x + bd-weights, one paired transfer per 2 tiles."""
                jt = work[i]
                nt = min(NT, nl - jt * NT)
                C = nt * 32
                if i % 2 == 0:
                    jt1 = work[i + 1] if i + 1 < n and work[i + 1] == jt + 1 \
                        else None
                    nt1 = min(NT, nl - (jt + 1) * NT) if jt1 is not None else 0
                    pitch = C + nt1 * 32
                    xt2 = xp.tile([96, NB * 1024], BF16, tag="xt2")
                    src = bass.AP(tensor=xin, offset=jt * NT * D,
                                  ap=[[nl * D, 96], [L * nl * D, NB],
                                      [1, pitch]])
                    nc.sync.dma_start(out=xt2[:, :NB * pitch], in_=src)
                    bdt2 = bdp.tile([128, 2 * BDC], F16, tag="bdt2")
                    nbd = 2 * BDC if jt1 is not None else BDC
                    nc.sync.dma_start(
                        out=bdt2[:, :nbd],
                        in_=bass.AP(tensor=bdw, offset=jt * BDC,
                                    ap=[[NTILES * BDC, 128], [1, nbd]]))
                    pair[i // 2] = {"xt2": xt2, "bdt2": bdt2, "pitch": pitch,
                                    "C0": C, "jt0": jt}
                p = pair[i // 2]
                off = 0 if i % 2 == 0 else p["C0"]
                return {"jt": jt, "nt": nt, "C": C, "p": p, "xoff": off,
                        "bdt": p["bdt2"], "bdoff": (i % 2) * BDC}

            def xtb(s, b):
                """Moving x slice for batch b of item s."""
                p = s["p"]
                base = b * p["pitch"] + s["xoff"]
                return p["xt2"][:, base:base + s["C"]]

            def s_mma(i):
                """P projection: 8 accumulating matmuls into ps48."""
                s = st[i]
                C = s["C"]
                ps48 = ps48p.tile([128, 512], F32, tag="ps48")
                for b in range(NB):
                    nc.tensor.matmul(
                        ps48[:, :C], cc_sb[:, 128 * b:128 * (b + 1)],
                        xtb(s, b), start=(b == 0), stop=(b == NB - 1))
                ps_sb = psb.tile([48, 512], F16, tag="ps_sb")
                nc.vector.tensor_copy(ps_sb[:, :C], ps48[:48, :C])
                s["ps_sb"] = ps_sb

            def s_transp(i):
                """PE-transpose P to [node*dim, (b, comp)] + negated mirror.

                pt_sb layout: region A [0,192): negated im comps at
                48c+m+6b; region B [192,384): transposed P at 192+48c+cp+6b.
                """
                s = st[i]
                C, nqt = s["C"], (s["nt"] + 3) // 4
                pt_ps = ptp.tile([128, 192], F16, tag="pt")
                pt_sb = ptsb.tile([128, 384], F16, tag="pt_sb")
                for c in range(nqt):
                    w = min(128, C - 128 * c)
                    nc.tensor.transpose(
                        pt_ps[:w, 48 * c:48 * c + 48],
                        s["ps_sb"][:, 128 * c:128 * c + w], id48_sb[:])
                    nc.vector.tensor_copy(
                        pt_sb[:w, 192 + 48 * c:192 + 48 * c + 48],
                        pt_ps[:w, 48 * c:48 * c + 48])
                    # negated im comps mirrored into region A (same 6-stride
                    # layout as region B so u2's moving AP spans both).
                    for m in range(3):
                        sH = 48 * c + 3 + m
                        nc.vector.tensor_scalar_mul(
                            pt_sb[:w, 48 * c + m:48 * c + m + 6 * (NB - 1) + 1:6],
                            pt_ps[:w, sH:sH + 6 * (NB - 1) + 1:6],
                            -1.0)
                s["pt_sb"] = pt_sb

            def s_mix(i):
                """Per-node complex mixing, 2 16-col matmuls per (quad, mode)."""
                s = st[i]
                nqt = (s["nt"] + 3) // 4
                bdt, bd0 = s["bdt"], s["bdoff"]
                pt_sb = s["pt_sb"]
                zq_ps = zqp.tile([128, 192], F32, tag="zq")
                for c in range(nqt):
                    for m in range(3):
                        u1 = bdt[:, bd0 + ((c * 3 + m) * 2) * 128:
                                 bd0 + ((c * 3 + m) * 2) * 128 + 128]
                        u2 = bdt[:, bd0 + ((c * 3 + m) * 2 + 1) * 128:
                                 bd0 + ((c * 3 + m) * 2 + 1) * 128 + 128]
                        if mixmerge:
                            # b-major zq: col 48c+6b+cp.  out order (j, b):
                            # j=0 -> re (cp=m), j=1 -> im (cp=3+m)
                            zout = _ap2(zq_ps, 128, 48 * c + m, 3, 2, 6, 8)
                            # u1 moving: j=0 Pre(m), j=1 Pim(m) (region B)
                            m1 = _ap2(pt_sb, 128, 192 + 48 * c + m, 3, 2, 6, 8)
                            # u2 moving: j=0 -Pim(m) (region A), j=1 Pre(m)
                            m2 = _ap2(pt_sb, 128, 48 * c + m, 192, 2, 6, 8)
                            nc.tensor.matmul(zout, u1, m1, start=True,
                                             stop=False)
                            nc.tensor.matmul(zout, u2, m2, start=False,
                                             stop=True)
                        else:
                            def pcols(cidx):
                                sH = 192 + 48 * c + cidx
                                return pt_sb[:, sH:sH + 6 * (NB - 1) + 1:6]

                            def zcols(cp):
                                sH = 48 * c + cp
                                return zq_ps[:, sH:sH + 6 * (NB - 1) + 1:6]

                            pn = pt_sb[:, 48 * c + m:
                                       48 * c + m + 6 * (NB - 1) + 1:6]
                            nc.tensor.matmul(zcols(m), u1, pcols(m),
                                             start=True, stop=False)
                            nc.tensor.matmul(zcols(3 + m), u1, pcols(3 + m),
                                             start=True, stop=False)
                            nc.tensor.matmul(zcols(m), u2, pn,
                                             start=False, stop=True)
                            nc.tensor.matmul(zcols(3 + m), u2, pcols(m),
                                             start=False, stop=True)
                zq_sb = zqsb.tile([128, 192], F16, tag="zq_sb")
                nc.vector.tensor_copy(zq_sb[:, :48 * nqt], zq_ps[:, :48 * nqt])
                s["zq_sb"] = zq_sb

            def s_ztt(i):
                """Transpose Z back to [(b, comp), (node, dim)]."""
                s = st[i]
                C, nqt = s["C"], (s["nt"] + 3) // 4
                zt_ps = ztp.tile([48, 512], F16, tag="zt")
                zt_sb = ztsb.tile([48, 512], F16, tag="zt_sb")
                for c in range(nqt):
                    w = min(128, C - 128 * c)
                    nc.tensor.transpose(
                        zt_ps[:, 128 * c:128 * c + w],
                        s["zq_sb"][:w, 48 * c:48 * c + 48], id128_sb[:w, :w])
                    nc.vector.tensor_copy(
                        zt_sb[:, 128 * c:128 * c + w],
                        zt_ps[:, 128 * c:128 * c + w])
                s["zt_sb"] = zt_sb

            def s_b(i):
                """Residual: res = M_res @ x + G @ Z, evac, store.

                Paired so the shared matp stationary is loaded once per pair;
                output DMA covers 2 tiles (s_b of the odd item issues it).
                """
                s = st[i]
                C, pr = s["C"], s["p"]
                pitch = pr["pitch"]
                if i % 2 == 0:
                    out2 = outp.tile([96, NB * 1024], BF16, tag="out2")
                    pr["out2"] = out2
                else:
                    out2 = pr["out2"]
                ooff = s["xoff"]
                for p in range(NB // 2):
                    b0, b1 = 2 * p, 2 * p + 1
                    r0 = resp.tile([128, 512], F32, tag="res0")
                    r1 = resp.tile([128, 512], F32, tag="res1")
                    nc.tensor.matmul(r0[:, :C], mat_sb[:], xtb(s, b0),
                                     start=True, stop=False)
                    nc.tensor.matmul(r1[:, :C], mat_sb[:], xtb(s, b1),
                                     start=True, stop=False)
                    nc.tensor.matmul(r0[:, :C],
                                     gt_sb[:, 128 * b0:128 * (b0 + 1)],
                                     s["zt_sb"][:, :C], start=False, stop=True)
                    nc.tensor.matmul(r1[:, :C],
                                     gt_sb[:, 128 * b1:128 * (b1 + 1)],
                                     s["zt_sb"][:, :C], start=False, stop=True)
                    nc.vector.tensor_copy(
                        out2[:, b0 * pitch + ooff:b0 * pitch + ooff + C],
                        r0[:96, :C])
                    nc.scalar.copy(
                        out=out2[:, b1 * pitch + ooff:b1 * pitch + ooff + C],
                        in_=r1[:96, :C])
                last = (i % 2 == 1) or (i + 1 >= n) or (work[i + 1] != s["jt"] + 1)
                if last:
                    dst = bass.AP(tensor=rout, offset=pr["jt0"] * NT * D,
                                  ap=[[nl * D, 96], [L * nl * D, NB],
                                      [1, pitch]])
                    nc.sync.dma_start(out=dst, in_=out2[:, :NB * pitch])
                    pair.pop(i // 2, None)

            if pipe:
                # software pipeline: load(i+2) | mma(i) | transp(i-1)
                #                    | mix(i-2) | b(i-3) | ztt(i-2)
                st[0] = s_load(0)
                st[1] = s_load(1)
                for i in range(n + 3):
                    if i + 2 < n:
                        st[i + 2] = s_load(i + 2)
                    if i < n:
                        s_mma(i)
                    if 0 <= i - 1 < n:
                        s_transp(i - 1)
                    if 0 <= i - 2 < n:
                        s_mix(i - 2)
                    if 0 <= i - 3 < n:
                        s_b(i - 3)
                        st.pop(i - 3)
                    if 0 <= i - 2 < n:
                        s_ztt(i - 2)
            else:
                st[0] = s_load(0)
                for i in range(n):
                    if i + 1 < n:
                        st[i + 1] = s_load(i + 1)
                    s_mma(i)
                    s_transp(i)
                    s_mix(i)
                    s_ztt(i)
                    s_b(i)
                    st.pop(i)
    nc.compile()
    return nc


from concourse.bass_utils import run_bass_kernel_spmd

_NC_CACHE = None


def kernel(x, Wq, bq, Wk, bk, Wv, bv, Wo, bo, W1, W2):
    """Full inputs -> full output res [8, 2000, 96, 32] float32."""
    global _NC_CACHE
    maps = shard_inputs({"x": np.asarray(x), "W1": np.asarray(W1),
                         "W2": np.asarray(W2), "Wq": np.asarray(Wq),
                         "Wo": np.asarray(Wo)})
    if _NC_CACHE is None:
        _NC_CACHE = build_kernel(reps=1)
    res = run_bass_kernel_spmd(_NC_CACHE, maps, list(range(NCORES)))
    rvals = [res.results[c]["rout"] for c in range(NCORES)]
    return unshard(rvals)
